# revision 9
# baseline (speedup 1.0000x reference)
"""BandSplit Trainium2 kernel: 8-core data-parallel over batch.

out[b,t,n,d] = rsqrt(ms + eps) * (x_band @ (norm_w * W)) + bias
with ms = sum(x_band^2)/(4*fn),  x_band = contiguous freq slices of X.

Math restructure (exact):
  rsqrt(ms + eps) = sqrt(4fn) / sqrt(ssum),  ssum = sum(x^2) + 4*fn*eps
  out = (1/sqrt(ssum)) * (x @ W2 + sqrt(ssum) * b)     [bias-row trick]
  where W2[n,p,:] = sqrt(4fn) * norm_w[n,p] * W[n,p,:]

Per core (one batch element, T=512 tokens, processed in 2 halves of 256):
  1. DMA X planes (t,f) natural layout; PE-transpose to (f,t).
  2. SBUF->SBUF DMAs reorder rows into band-major packed chunks (XtB),
     slot-aligned so matmul base_partition is in {0,32,64,96}.
  3. ssum via matmuls with a 0/1 selection matrix on squared transposed X.
  4. Bias rows of XtB get sqrt(ssum[t,n]) so the eviction scale cancels.
  5. Per band: 1-3 float32r matmuls (K<=128, M=128 tokens, N=384).
  6. Evict PSUM->SBUF with per-token scale 1/sqrt(ssum); DMA out.
"""

import math
import numpy as np

# ---------------- problem geometry (hardcoded, matches reference) ----------
SR, N_FFT, D = 44100, 2048, 384
RANGES = [(1000, 2), (2000, 4), (4000, 12), (8000, 24), (16000, 48)]


def _compute_bands(sr=SR, n_fft=N_FFT):
    hz_per_bin = sr / n_fft
    max_bin = n_fft // 2 + 1
    boundaries = [0]
    for hi_hz, bins in RANGES:
        hi_bin = math.floor(hi_hz / hz_per_bin)
        while boundaries[-1] + bins <= hi_bin and boundaries[-1] + bins <= max_bin:
            boundaries.append(boundaries[-1] + bins)
    if boundaries[-1] < max_bin:
        remaining = max_bin - boundaries[-1]
        step = math.ceil(remaining / 6)
        b = boundaries[-1]
        while b + step < max_bin:
            b += step
            boundaries.append(b)
        boundaries.append(max_bin)
    return [(boundaries[i], boundaries[i + 1]) for i in range(len(boundaries) - 1)]


BANDS = _compute_bands()
N_BANDS = len(BANDS)
assert N_BANDS == 62
FN = [r - l for l, r in BANDS]
MAXF = max(FN)
F_BINS = N_FFT // 2 + 1  # 1025
EPS = 1e-8
B, C, T = 8, 2, 512
TH = 256  # half of T processed per pass
NH = T // TH
TT = 128  # token tile (matmul M)
NTT = TH // TT  # token tiles per half
NPLANE = 4  # (c, ri) combinations, g = 2*c + ri
NFC = (F_BINS + 127) // 128  # 9 freq chunks of transposed X

# ---------------- row layout ----------------------------------------------
# logical row j of band n: j=0 -> bias; j=1+g*fn+k -> plane g, freq l+k.
# Bands are split into segments; each segment must be placed at a base
# partition legal for the PE: size<=32 -> {0,32,64,96}; <=64 -> {0,64};
# else base 0.


def _seg_sizes(fn):
    rows = 1 + 4 * fn
    if rows <= 64:
        return [rows]
    if rows <= 128:
        # only 97 occurs; split into two 64-slot segments
        h = (rows + 1) // 2
        return [h, rows - h]
    return [128, rows - 128]


def _plan():
    seg_sizes = {n: _seg_sizes(FN[n]) for n in range(N_BANDS)}
    chunks = []  # per chunk: list of (start, end)

    def try_place(ci, sz):
        bases = [0, 32, 64, 96] if sz <= 32 else ([0, 64] if sz <= 64 else [0])
        for bse in bases:
            if bse + sz <= 128 and all(
                e <= bse or s >= bse + sz for (s, e) in chunks[ci]
            ):
                return bse
        return None

    order = sorted(
        [(n, si, sz) for n in range(N_BANDS) for si, sz in enumerate(seg_sizes[n])],
        key=lambda x: -x[2],
    )
    place = {}
    for n, si, sz in order:
        for ci in range(len(chunks)):
            bse = try_place(ci, sz)
            if bse is not None:
                place[(n, si)] = (ci, bse, sz)
                chunks[ci].append((bse, bse + sz))
                break
        else:
            place[(n, si)] = (len(chunks), 0, sz)
            chunks.append([(0, sz)])
    nchunk = len(chunks)

    # W2 column blocks: segments with disjoint row intervals can share one.
    colblocks = []  # per block: list of (start, end)
    cb_of = {}
    for n, si, sz in order:
        _, r0, _ = place[(n, si)]
        for cbi in range(len(colblocks)):
            if all(e <= r0 or s >= r0 + sz for (s, e) in colblocks[cbi]):
                cb_of[(n, si)] = cbi
                colblocks[cbi].append((r0, r0 + sz))
                break
        else:
            cb_of[(n, si)] = len(colblocks)
            colblocks.append([(r0, r0 + sz)])
    ncolb = len(colblocks)

    def locate(n, j):
        off = 0
        for si, sz in enumerate(seg_sizes[n]):
            if j < off + sz:
                ci, r0, _ = place[(n, si)]
                return ci, r0 + (j - off)
            off += sz
        raise AssertionError

    # matmul segments per band: (chunk, row0, klen, colblock)
    segs = []
    for n in range(N_BANDS):
        segs.append(
            [place[(n, si)] + (cb_of[(n, si)],) for si in range(len(seg_sizes[n]))]
        )
    bias_slot = [locate(n, 0) for n in range(N_BANDS)]

    # reorder pieces per plane: (src_c, src_p0, dst_chunk, dst_row, ln)
    pieces = [[] for _ in range(NPLANE)]
    for n in range(N_BANDS):
        l, r = BANDS[n]
        fn = FN[n]
        sizes = seg_sizes[n]
        bnd = []
        off = 0
        for sz in sizes:
            off += sz
            bnd.append(off)
        for g in range(NPLANE):
            f = l
            while f < r:
                j = 1 + g * fn + (f - l)
                ch, row = locate(n, j)
                nb = min(x for x in bnd if x > j)
                dst_run = nb - j
                src_run = 128 - (f % 128)
                ln = min(r - f, dst_run, src_run)
                pieces[g].append((f // 128, f % 128, ch, row, ln))
                f += ln
    # logical-row map for host W2 packing: (n, j) -> (row, colblock)
    w2map = []
    for n in range(N_BANDS):
        rows = 1 + 4 * FN[n]
        m = []
        off = 0
        for si, sz in enumerate(seg_sizes[n]):
            _, r0, _ = place[(n, si)]
            cb = cb_of[(n, si)]
            for k in range(sz):
                m.append((r0 + k, cb))
            off += sz
        assert len(m) == rows
        w2map.append(m)
    return nchunk, ncolb, segs, bias_slot, pieces, w2map


NCHUNK, NCOLB, SEGS, BIAS_SLOT, PIECES, W2MAP = _plan()

# ---------------- host-side constant prep ---------------------------------


def _prep_consts(norm_w, W, b):
    w2sb = np.zeros((128, NCOLB * D), np.float32)
    for n in range(N_BANDS):
        fn = FN[n]
        s = math.sqrt(4.0 * fn)
        row, cb = W2MAP[n][0]
        w2sb[row, cb * D:(cb + 1) * D] = b[n]
        w2rows = (s * norm_w[n][:, None] * W[n]).astype(np.float32)  # (216, 384)
        for g in range(NPLANE):
            for k in range(fn):
                row, cb = W2MAP[n][1 + g * fn + k]
                w2sb[row, cb * D:(cb + 1) * D] = w2rows[g * MAXF + k]
    smat = np.zeros((128, NFC * N_BANDS), np.float32)
    for n, (l, r) in enumerate(BANDS):
        for f in range(l, r):
            smat[f % 128, (f // 128) * N_BANDS + n] = 1.0
    epsv = (4.0 * np.asarray(FN, np.float64) * EPS).astype(np.float32).reshape(
        N_BANDS, 1
    )
    idt = np.eye(128, dtype=np.float32)
    return w2sb, smat, epsv, idt


# ---------------- bass kernel builder -------------------------------------

_BUILT = {}


def _build():
    if "nc" in _BUILT:
        return _BUILT["nc"]
    from contextlib import ExitStack
    import concourse.bacc as bacc
    import concourse.mybir as mybir
    from concourse import tile

    f32 = mybir.dt.float32
    f32r = mybir.dt.float32r

    nc = bacc.Bacc(None, target_bir_lowering=False)
    x_re = nc.declare_dram_parameter("X_real", [C, T, F_BINS], f32, isOutput=False)
    x_im = nc.declare_dram_parameter("X_imag", [C, T, F_BINS], f32, isOutput=False)
    w2_e = nc.declare_dram_parameter("W2", [128, NCOLB * D], f32, isOutput=False)
    s_e = nc.declare_dram_parameter("SMAT", [128, NFC * N_BANDS], f32, isOutput=False)
    eps_e = nc.declare_dram_parameter("EPSV", [N_BANDS, 1], f32, isOutput=False)
    id_e = nc.declare_dram_parameter("IDT", [128, 128], f32, isOutput=False)
    out_e = nc.declare_dram_parameter("out", [T, N_BANDS, D], f32, isOutput=True)

    GSZ = 4  # bands per output staging group

    with tile.TileContext(nc) as tc, ExitStack() as ctx:
        const = ctx.enter_context(tc.tile_pool(name="const", bufs=1))
        xtbp = ctx.enter_context(tc.tile_pool(name="xtbp", bufs=1))
        xpool = ctx.enter_context(tc.tile_pool(name="xinp", bufs=2))
        gpool = ctx.enter_context(tc.tile_pool(name="xtgp", bufs=2))
        qpool = ctx.enter_context(tc.tile_pool(name="sqp", bufs=2))
        mspool = ctx.enter_context(tc.tile_pool(name="msv", bufs=2))
        spool = ctx.enter_context(tc.tile_pool(name="stagep", bufs=2))
        trps = ctx.enter_context(tc.tile_pool(name="trp", bufs=3, space="PSUM"))
        mmps = ctx.enter_context(tc.tile_pool(name="mmp", bufs=4, space="PSUM"))
        msps = ctx.enter_context(tc.tile_pool(name="msp", bufs=1, space="PSUM"))

        w2sb = const.tile([128, NCOLB * D], f32r)
        idsb = const.tile([128, 128], f32)
        ssb = const.tile([128, NFC * N_BANDS], f32)
        epsb = const.tile([N_BANDS, 1], f32)
        nc.sync.dma_start(out=idsb[:], in_=id_e[:])
        nc.sync.dma_start(out=ssb[:], in_=s_e[:])
        nc.sync.dma_start(out=epsb[:], in_=eps_e[:])
        wr_pool = ctx.enter_context(tc.tile_pool(name="wrp", bufs=2))
        for m in range(NCOLB):
            wstage = wr_pool.tile([128, D], f32, tag="wstage")
            nc.sync.dma_start(out=wstage[:], in_=w2_e[:, m * D:(m + 1) * D])
            nc.vector.tensor_copy(w2sb[:, m * D:(m + 1) * D], wstage[:])

        xtb = [xtbp.tile([128, TH], f32r, name=f"xtb{m}", tag=f"xtb{m}") for m in range(NCHUNK)]

        for h in range(NH):
            t_h = h * TH
            for g in range(NPLANE):
                xtg = gpool.tile([128, NFC, TH], f32r, tag="xtg")
                nc.gpsimd.memset(xtg[:, NFC - 1, :].bitcast(mybir.dt.uint32), 0)
                xsrc = x_re if g % 2 == 0 else x_im
                ci = g // 2
                for tt in range(NTT):
                    t0 = t_h + tt * TT
                    xin = xpool.tile([128, F_BINS], f32, tag="xin")
                    nc.sync.dma_start(out=xin[:], in_=xsrc[ci, t0:t0 + TT, :])
                    for c in range(NFC):
                        wsz = min(128, F_BINS - c * 128)
                        ptr = trps.tile([128, 128], f32, tag="trp")
                        nc.tensor.transpose(
                            ptr[0:wsz, 0:128], xin[:, c * 128:c * 128 + wsz], idsb[:]
                        )
                        nc.vector.tensor_copy(
                            xtg[0:wsz, c, tt * TT:(tt + 1) * TT], ptr[0:wsz, 0:128]
                        )
                psum_ms = msps.tile([N_BANDS, TH], f32, tag="msp")
                for c in range(NFC):
                    sqc = qpool.tile([128, TH], f32, tag="sqc")
                    nc.vector.tensor_tensor(
                        out=sqc[:], in0=xtg[:, c, :].bitcast(f32),
                        in1=xtg[:, c, :].bitcast(f32),
                        op=mybir.AluOpType.mult,
                    )
                    nc.tensor.matmul(
                        psum_ms[:],
                        lhsT=ssb[:, c * N_BANDS:(c + 1) * N_BANDS],
                        rhs=sqc[:],
                        start=(g == 0 and c == 0),
                        stop=(g == NPLANE - 1 and c == NFC - 1),
                    )
                for (src_c, src_p0, dch, drow, ln) in PIECES[g]:
                    nc.sync.dma_start(
                        out=xtb[dch][drow:drow + ln, :],
                        in_=xtg[src_p0:src_p0 + ln, src_c, :],
                    )

            ssum_t = mspool.tile([N_BANDS, TH], f32, tag="ssum")
            sqrt_t = mspool.tile([N_BANDS, TH], f32r, tag="sqrt")
            rs = mspool.tile([128, NTT, N_BANDS], f32, tag="rs")
            nc.vector.tensor_scalar_add(out=ssum_t[:], in0=psum_ms[:], scalar1=epsb[:])
            nc.scalar.activation(
                out=sqrt_t[:], in_=ssum_t[:], func=mybir.ActivationFunctionType.Sqrt
            )
            for n in range(N_BANDS):
                ch, row = BIAS_SLOT[n]
                nc.sync.dma_start(
                    out=xtb[ch][row:row + 1, :], in_=sqrt_t[n:n + 1, :]
                )
            for tt in range(NTT):
                ptr = trps.tile([128, 128], f32, tag="trp")
                nc.tensor.transpose(
                    ptr[0:128, 0:N_BANDS],
                    sqrt_t[:, tt * TT:(tt + 1) * TT].bitcast(f32),
                    idsb[0:N_BANDS, 0:N_BANDS],
                )
                nc.vector.reciprocal(rs[:, tt, :], ptr[0:128, 0:N_BANDS])

            for tt in range(NTT):
                t0 = t_h + tt * TT
                for n0 in range(0, N_BANDS, GSZ):
                    gn = min(GSZ, N_BANDS - n0)
                    stage = spool.tile([128, GSZ * D], f32, tag="stage")
                    for n in range(n0, n0 + gn):
                        pmm = mmps.tile([128, D], f32, tag="mmp")
                        nseg = len(SEGS[n])
                        for si, (ch, row0, klen, cb) in enumerate(SEGS[n]):
                            nc.tensor.matmul(
                                pmm[:],
                                lhsT=xtb[ch][
                                    row0:row0 + klen, tt * TT:(tt + 1) * TT
                                ],
                                rhs=w2sb[
                                    row0:row0 + klen, cb * D:(cb + 1) * D
                                ],
                                start=(si == 0),
                                stop=(si == nseg - 1),
                                tile_position=(row0, 0),
                            )
                        slot = stage[:, (n - n0) * D:(n - n0 + 1) * D]
                        if n % 2 == 0:
                            nc.vector.tensor_scalar_mul(slot, pmm[:], rs[:, tt, n:n + 1])
                        else:
                            nc.scalar.mul(slot, pmm[:], rs[:, tt, n:n + 1])
                    nc.sync.dma_start(
                        out=out_e[t0:t0 + TT, n0:n0 + gn, :],
                        in_=stage[:, 0:gn * D].rearrange("p (n d) -> p n d", n=gn),
                    )

    nc.finalize()
    _BUILT["nc"] = nc
    return nc


# ---------------- entry points --------------------------------------------


def _run(in_maps, trace=False):
    from concourse.bass_utils import run_bass_kernel_spmd

    nc = _build()
    return run_bass_kernel_spmd(nc, in_maps, core_ids=list(range(8)), trace=trace)


def _make_in_maps(X_real, X_imag, norm_w, W, b):
    X_real = np.ascontiguousarray(np.asarray(X_real, np.float32))
    X_imag = np.ascontiguousarray(np.asarray(X_imag, np.float32))
    w2sb, smat, epsv, idt = _prep_consts(
        np.asarray(norm_w, np.float32), np.asarray(W, np.float32),
        np.asarray(b, np.float32),
    )
    return [
        {
            "X_real": X_real[i],
            "X_imag": X_imag[i],
            "W2": w2sb,
            "SMAT": smat,
            "EPSV": epsv,
            "IDT": idt,
        }
        for i in range(B)
    ]


def kernel(X_real, X_imag, norm_w, W, b):
    res = _run(_make_in_maps(X_real, X_imag, norm_w, W, b), trace=False)
    return np.stack([res.results[i]["out"] for i in range(B)]).astype(np.float32)


def kernel_profiled(X_real, X_imag, norm_w, W, b):
    res = _run(_make_in_maps(X_real, X_imag, norm_w, W, b), trace=True)
    out = np.stack([res.results[i]["out"] for i in range(B)]).astype(np.float32)
    return out, res


if __name__ == "__main__":
    print(f"NCHUNK={NCHUNK} NCOLB={NCOLB}")
    total_rows = sum(1 + 4 * f for f in FN)
    print(f"rows={total_rows} capacity={NCHUNK * 128}")
    print(f"pieces per plane: {[len(p) for p in PIECES]}")
    print(f"matmul segs per ttile: {sum(len(s) for s in SEGS)}")
    kb = (NCHUNK * TH * 4 + NCOLB * D * 4 + NFC * TH * 4 * 2 * 2 + F_BINS * 4 * 2
          + TH * 4 * 2 * 2 + 4 * D * 4 * 2) / 1024
    print(f"approx SBUF per partition: {kb:.0f} KB")


# revision 10
# speedup vs baseline: 6.2871x; 6.2871x over previous
"""BandSplit Trainium2 kernel: 8-core data-parallel over batch.

out[b,t,n,d] = rsqrt(ms + eps) * (x_band @ (norm_w * W)) + bias
with ms = sum(x_band^2)/(4*fn),  x_band = contiguous freq slices of X.

Math restructure (exact):
  rsqrt(ms + eps) = sqrt(4fn) / sqrt(ssum),  ssum = sum(x^2) + 4*fn*eps
  out = (1/sqrt(ssum)) * (x @ W2 + sqrt(ssum) * b)     [bias-row trick]
  where W2[n,p,:] = sqrt(4fn) * norm_w[n,p] * W[n,p,:]

Per core (one batch element, T=512 tokens, processed in 2 halves of 256):
  1. DMA X planes (t,f) natural layout; PE-transpose to (f,t).
  2. SBUF->SBUF DMAs reorder rows into band-major packed chunks (XtB),
     slot-aligned so matmul base_partition is in {0,32,64,96}.
  3. ssum via matmuls with a 0/1 selection matrix on squared transposed X.
  4. Bias rows of XtB get sqrt(ssum[t,n]) so the eviction scale cancels.
  5. Per band: 1-3 float32r matmuls (K<=128, M=128 tokens, N=384).
  6. Evict PSUM->SBUF with per-token scale 1/sqrt(ssum); DMA out.
"""

import math
import numpy as np

# ---------------- problem geometry (hardcoded, matches reference) ----------
SR, N_FFT, D = 44100, 2048, 384
RANGES = [(1000, 2), (2000, 4), (4000, 12), (8000, 24), (16000, 48)]


def _compute_bands(sr=SR, n_fft=N_FFT):
    hz_per_bin = sr / n_fft
    max_bin = n_fft // 2 + 1
    boundaries = [0]
    for hi_hz, bins in RANGES:
        hi_bin = math.floor(hi_hz / hz_per_bin)
        while boundaries[-1] + bins <= hi_bin and boundaries[-1] + bins <= max_bin:
            boundaries.append(boundaries[-1] + bins)
    if boundaries[-1] < max_bin:
        remaining = max_bin - boundaries[-1]
        step = math.ceil(remaining / 6)
        b = boundaries[-1]
        while b + step < max_bin:
            b += step
            boundaries.append(b)
        boundaries.append(max_bin)
    return [(boundaries[i], boundaries[i + 1]) for i in range(len(boundaries) - 1)]


BANDS = _compute_bands()
N_BANDS = len(BANDS)
assert N_BANDS == 62
FN = [r - l for l, r in BANDS]
MAXF = max(FN)
F_BINS = N_FFT // 2 + 1  # 1025
EPS = 1e-8
B, C, T = 8, 2, 512
TH = 256  # half of T processed per pass
NH = T // TH
TT = 128  # token tile (matmul M)
NTT = TH // TT  # token tiles per half
NPLANE = 4  # (c, ri) combinations, g = 2*c + ri
NFC = (F_BINS + 127) // 128  # 9 freq chunks of transposed X

# ---------------- row layout ----------------------------------------------
# logical row j of band n: j=0 -> bias; j=1+g*fn+k -> plane g, freq l+k.
# Bands are split into segments; each segment must be placed at a base
# partition legal for the PE: size<=32 -> {0,32,64,96}; <=64 -> {0,64};
# else base 0.


def _seg_sizes(fn):
    rows = 1 + 4 * fn
    if rows <= 64:
        return [rows]
    if rows <= 128:
        # only 97 occurs; split into two 64-slot segments
        h = (rows + 1) // 2
        return [h, rows - h]
    return [128, rows - 128]


def _plan():
    seg_sizes = {n: _seg_sizes(FN[n]) for n in range(N_BANDS)}
    chunks = []  # per chunk: list of (start, end)

    def try_place(ci, sz):
        bases = [0, 32, 64, 96] if sz <= 32 else ([0, 64] if sz <= 64 else [0])
        for bse in bases:
            if bse + sz <= 128 and all(
                e <= bse or s >= bse + sz for (s, e) in chunks[ci]
            ):
                return bse
        return None

    order = sorted(
        [(n, si, sz) for n in range(N_BANDS) for si, sz in enumerate(seg_sizes[n])],
        key=lambda x: -x[2],
    )
    place = {}
    for n, si, sz in order:
        for ci in range(len(chunks)):
            bse = try_place(ci, sz)
            if bse is not None:
                place[(n, si)] = (ci, bse, sz)
                chunks[ci].append((bse, bse + sz))
                break
        else:
            place[(n, si)] = (len(chunks), 0, sz)
            chunks.append([(0, sz)])
    nchunk = len(chunks)

    # W2 column blocks: segments with disjoint row intervals can share one.
    colblocks = []  # per block: list of (start, end)
    cb_of = {}
    for n, si, sz in order:
        _, r0, _ = place[(n, si)]
        for cbi in range(len(colblocks)):
            if all(e <= r0 or s >= r0 + sz for (s, e) in colblocks[cbi]):
                cb_of[(n, si)] = cbi
                colblocks[cbi].append((r0, r0 + sz))
                break
        else:
            cb_of[(n, si)] = len(colblocks)
            colblocks.append([(r0, r0 + sz)])
    ncolb = len(colblocks)

    def locate(n, j):
        off = 0
        for si, sz in enumerate(seg_sizes[n]):
            if j < off + sz:
                ci, r0, _ = place[(n, si)]
                return ci, r0 + (j - off)
            off += sz
        raise AssertionError

    # matmul segments per band: (chunk, row0, klen, colblock)
    segs = []
    for n in range(N_BANDS):
        segs.append(
            [place[(n, si)] + (cb_of[(n, si)],) for si in range(len(seg_sizes[n]))]
        )
    bias_slot = [locate(n, 0) for n in range(N_BANDS)]

    # reorder pieces per plane: (src_c, src_p0, dst_chunk, dst_row, ln)
    pieces = [[] for _ in range(NPLANE)]
    for n in range(N_BANDS):
        l, r = BANDS[n]
        fn = FN[n]
        sizes = seg_sizes[n]
        bnd = []
        off = 0
        for sz in sizes:
            off += sz
            bnd.append(off)
        for g in range(NPLANE):
            f = l
            while f < r:
                j = 1 + g * fn + (f - l)
                ch, row = locate(n, j)
                nb = min(x for x in bnd if x > j)
                dst_run = nb - j
                src_run = 128 - (f % 128)
                ln = min(r - f, dst_run, src_run)
                pieces[g].append((f // 128, f % 128, ch, row, ln))
                f += ln
    # logical-row map for host W2 packing: (n, j) -> (row, colblock)
    w2map = []
    for n in range(N_BANDS):
        rows = 1 + 4 * FN[n]
        m = []
        off = 0
        for si, sz in enumerate(seg_sizes[n]):
            _, r0, _ = place[(n, si)]
            cb = cb_of[(n, si)]
            for k in range(sz):
                m.append((r0 + k, cb))
            off += sz
        assert len(m) == rows
        w2map.append(m)
    return nchunk, ncolb, segs, bias_slot, pieces, w2map


NCHUNK, NCOLB, SEGS, BIAS_SLOT, PIECES, W2MAP = _plan()

# ---------------- host-side constant prep ---------------------------------


def _prep_consts(norm_w, W, b):
    w2sb = np.zeros((128, NCOLB * D), np.float32)
    for n in range(N_BANDS):
        fn = FN[n]
        s = math.sqrt(4.0 * fn)
        row, cb = W2MAP[n][0]
        w2sb[row, cb * D:(cb + 1) * D] = b[n]
        w2rows = (s * norm_w[n][:, None] * W[n]).astype(np.float32)  # (216, 384)
        for g in range(NPLANE):
            for k in range(fn):
                row, cb = W2MAP[n][1 + g * fn + k]
                w2sb[row, cb * D:(cb + 1) * D] = w2rows[g * MAXF + k]
    smat = np.zeros((128, NFC * N_BANDS), np.float32)
    for n, (l, r) in enumerate(BANDS):
        for f in range(l, r):
            smat[f % 128, (f // 128) * N_BANDS + n] = 1.0
    epsv = (4.0 * np.asarray(FN, np.float64) * EPS).astype(np.float32).reshape(
        N_BANDS, 1
    )
    idt = np.eye(128, dtype=np.float32)
    return w2sb, smat, epsv, idt


# ---------------- bass kernel builder -------------------------------------

_BUILT = {}


def _build():
    if "nc" in _BUILT:
        return _BUILT["nc"]
    from contextlib import ExitStack
    import concourse.bacc as bacc
    import concourse.mybir as mybir
    from concourse import tile

    f32 = mybir.dt.float32
    f32r = mybir.dt.float32r

    nc = bacc.Bacc(None, target_bir_lowering=False)
    x_re = nc.declare_dram_parameter("X_real", [C, T, F_BINS], f32, isOutput=False)
    x_im = nc.declare_dram_parameter("X_imag", [C, T, F_BINS], f32, isOutput=False)
    w2_e = nc.declare_dram_parameter("W2", [128, NCOLB * D], f32, isOutput=False)
    s_e = nc.declare_dram_parameter("SMAT", [128, NFC * N_BANDS], f32, isOutput=False)
    eps_e = nc.declare_dram_parameter("EPSV", [N_BANDS, 1], f32, isOutput=False)
    id_e = nc.declare_dram_parameter("IDT", [128, 128], f32, isOutput=False)
    out_e = nc.declare_dram_parameter("out", [T, N_BANDS, D], f32, isOutput=True)

    GSZ = 4  # bands per output staging group

    with tile.TileContext(nc) as tc, ExitStack() as ctx:
        const = ctx.enter_context(tc.tile_pool(name="const", bufs=1))
        xtbp = ctx.enter_context(tc.tile_pool(name="xtbp", bufs=1))
        xpool = ctx.enter_context(tc.tile_pool(name="xinp", bufs=2))
        gpool = ctx.enter_context(tc.tile_pool(name="xtgp", bufs=2))
        qpool = ctx.enter_context(tc.tile_pool(name="sqp", bufs=2))
        mspool = ctx.enter_context(tc.tile_pool(name="msv", bufs=2))
        spool = ctx.enter_context(tc.tile_pool(name="stagep", bufs=2))
        trps = ctx.enter_context(tc.tile_pool(name="trp", bufs=3, space="PSUM"))
        mmps = ctx.enter_context(tc.tile_pool(name="mmp", bufs=4, space="PSUM"))
        msps = ctx.enter_context(tc.tile_pool(name="msp", bufs=1, space="PSUM"))

        w2sb = const.tile([128, NCOLB * D], f32r)
        idsb = const.tile([128, 128], f32)
        ssb = const.tile([128, NFC * N_BANDS], f32)
        epsb = const.tile([N_BANDS, 1], f32)
        nc.sync.dma_start(out=idsb[:], in_=id_e[:])
        nc.sync.dma_start(out=ssb[:], in_=s_e[:])
        nc.sync.dma_start(out=epsb[:], in_=eps_e[:])
        wr_pool = ctx.enter_context(tc.tile_pool(name="wrp", bufs=2))
        for m in range(NCOLB):
            wstage = wr_pool.tile([128, D], f32, tag="wstage")
            nc.sync.dma_start(out=wstage[:], in_=w2_e[:, m * D:(m + 1) * D])
            nc.vector.tensor_copy(w2sb[:, m * D:(m + 1) * D], wstage[:])

        xtb = [xtbp.tile([128, TH], f32r, name=f"xtb{m}", tag=f"xtb{m}") for m in range(NCHUNK)]

        for h in range(NH):
            t_h = h * TH
            for g in range(NPLANE):
                xtg = gpool.tile([128, NFC, TH], f32r, tag="xtg")
                nc.gpsimd.memset(xtg[:, NFC - 1, :].bitcast(mybir.dt.uint32), 0)
                xsrc = x_re if g % 2 == 0 else x_im
                ci = g // 2
                for tt in range(NTT):
                    t0 = t_h + tt * TT
                    xin = xpool.tile([128, F_BINS], f32, tag="xin")
                    nc.sync.dma_start(out=xin[:], in_=xsrc[ci, t0:t0 + TT, :])
                    for c in range(NFC):
                        wsz = min(128, F_BINS - c * 128)
                        ptr = trps.tile([128, 128], f32, tag="trp")
                        nc.tensor.transpose(
                            ptr[0:wsz, 0:128], xin[:, c * 128:c * 128 + wsz], idsb[:]
                        )
                        nc.vector.tensor_copy(
                            xtg[0:wsz, c, tt * TT:(tt + 1) * TT], ptr[0:wsz, 0:128]
                        )
                psum_ms = msps.tile([N_BANDS, TH], f32, tag="msp")
                for c in range(NFC):
                    sqc = qpool.tile([128, TH], f32, tag="sqc")
                    nc.vector.tensor_tensor(
                        out=sqc[:], in0=xtg[:, c, :].bitcast(f32),
                        in1=xtg[:, c, :].bitcast(f32),
                        op=mybir.AluOpType.mult,
                    )
                    nc.tensor.matmul(
                        psum_ms[:],
                        lhsT=ssb[:, c * N_BANDS:(c + 1) * N_BANDS],
                        rhs=sqc[:],
                        start=(g == 0 and c == 0),
                        stop=(g == NPLANE - 1 and c == NFC - 1),
                    )
                for (src_c, src_p0, dch, drow, ln) in PIECES[g]:
                    nc.sync.dma_start(
                        out=xtb[dch][drow:drow + ln, :],
                        in_=xtg[src_p0:src_p0 + ln, src_c, :],
                    )

            ssum_t = mspool.tile([N_BANDS, TH], f32, tag="ssum")
            sqrt_t = mspool.tile([N_BANDS, TH], f32r, tag="sqrt")
            rs = mspool.tile([128, NTT, N_BANDS], f32, tag="rs")
            nc.vector.tensor_scalar_add(out=ssum_t[:], in0=psum_ms[:], scalar1=epsb[:])
            nc.scalar.activation(
                out=sqrt_t[:], in_=ssum_t[:], func=mybir.ActivationFunctionType.Sqrt
            )
            for n in range(N_BANDS):
                ch, row = BIAS_SLOT[n]
                nc.sync.dma_start(
                    out=xtb[ch][row:row + 1, :], in_=sqrt_t[n:n + 1, :]
                )
            for tt in range(NTT):
                ptr = trps.tile([128, 128], f32, tag="trp")
                nc.tensor.transpose(
                    ptr[0:128, 0:N_BANDS],
                    sqrt_t[:, tt * TT:(tt + 1) * TT].bitcast(f32),
                    idsb[0:N_BANDS, 0:N_BANDS],
                )
                nc.vector.reciprocal(rs[:, tt, :], ptr[0:128, 0:N_BANDS])

            for tt in range(NTT):
                t0 = t_h + tt * TT
                for n0 in range(0, N_BANDS, GSZ):
                    gn = min(GSZ, N_BANDS - n0)
                    stage = spool.tile([128, GSZ * D], f32, tag="stage")
                    for n in range(n0, n0 + gn):
                        pmm = mmps.tile([128, D], f32, tag="mmp")
                        nseg = len(SEGS[n])
                        for si, (ch, row0, klen, cb) in enumerate(SEGS[n]):
                            nc.tensor.matmul(
                                pmm[:],
                                lhsT=xtb[ch][
                                    row0:row0 + klen, tt * TT:(tt + 1) * TT
                                ],
                                rhs=w2sb[
                                    row0:row0 + klen, cb * D:(cb + 1) * D
                                ],
                                start=(si == 0),
                                stop=(si == nseg - 1),
                                tile_position=(row0, 0),
                            )
                        slot = stage[:, (n - n0) * D:(n - n0 + 1) * D]
                        if n % 2 == 0:
                            nc.vector.tensor_scalar_mul(slot, pmm[:], rs[:, tt, n:n + 1])
                        else:
                            nc.scalar.mul(slot, pmm[:], rs[:, tt, n:n + 1])
                    nc.sync.dma_start(
                        out=out_e[t0:t0 + TT, n0:n0 + gn, :],
                        in_=stage[:, 0:gn * D].rearrange("p (n d) -> p n d", n=gn),
                    )

    nc.finalize()
    _BUILT["nc"] = nc
    return nc


# ---------------- entry points --------------------------------------------


def _run(in_maps, trace=False):
    from concourse.bass_utils import run_bass_kernel_spmd

    nc = _build()
    return run_bass_kernel_spmd(nc, in_maps, core_ids=list(range(8)), trace=trace)


def _run_traced(in_maps, tmpdir=None):
    from concourse.bass_utils import run_bass_kernel_spmd

    nc = _build()
    return run_bass_kernel_spmd(
        nc, in_maps, core_ids=list(range(8)), trace=True, tmpdir=tmpdir
    )


def _make_in_maps(X_real, X_imag, norm_w, W, b):
    X_real = np.ascontiguousarray(np.asarray(X_real, np.float32))
    X_imag = np.ascontiguousarray(np.asarray(X_imag, np.float32))
    w2sb, smat, epsv, idt = _prep_consts(
        np.asarray(norm_w, np.float32), np.asarray(W, np.float32),
        np.asarray(b, np.float32),
    )
    return [
        {
            "X_real": X_real[i],
            "X_imag": X_imag[i],
            "W2": w2sb,
            "SMAT": smat,
            "EPSV": epsv,
            "IDT": idt,
        }
        for i in range(B)
    ]


def kernel(X_real, X_imag, norm_w, W, b):
    res = _run(_make_in_maps(X_real, X_imag, norm_w, W, b), trace=False)
    return np.stack([res.results[i]["out"] for i in range(B)]).astype(np.float32)


def kernel_profiled(X_real, X_imag, norm_w, W, b):
    res = _run(_make_in_maps(X_real, X_imag, norm_w, W, b), trace=True)
    out = np.stack([res.results[i]["out"] for i in range(B)]).astype(np.float32)
    return out, res


if __name__ == "__main__":
    print(f"NCHUNK={NCHUNK} NCOLB={NCOLB}")
    total_rows = sum(1 + 4 * f for f in FN)
    print(f"rows={total_rows} capacity={NCHUNK * 128}")
    print(f"pieces per plane: {[len(p) for p in PIECES]}")
    print(f"matmul segs per ttile: {sum(len(s) for s in SEGS)}")
    kb = (NCHUNK * TH * 4 + NCOLB * D * 4 + NFC * TH * 4 * 2 * 2 + F_BINS * 4 * 2
          + TH * 4 * 2 * 2 + 4 * D * 4 * 2) / 1024
    print(f"approx SBUF per partition: {kb:.0f} KB")


# revision 19
# speedup vs baseline: 15.5156x; 2.4678x over previous
"""BandSplit Trainium2 kernel: 8-core data-parallel over batch.

out[b,t,n,d] = rsqrt(ms + eps) * (x_band @ (norm_w * W)) + bias
with ms = sum(x_band^2)/(4*fn),  x_band = contiguous freq slices of X.

Math restructure (exact):
  rsqrt(ms + eps) = sqrt(4fn) / sqrt(ssum),  ssum = sum(x^2) + 4*fn*eps
  out = (1/sqrt(ssum)) * (x @ W2 + sqrt(ssum) * b)     [bias-row trick]
  where W2[n,p,:] = sqrt(4fn) * norm_w[n,p] * W[n,p,:]

Per core (one batch element, T=512 tokens, 4 passes of 128):
  1. DMA X planes into natural layout (t part, f free).
  2. ssum per band via one fused multiply-reduce per band (eps as initial).
  3. Free-axis gather (on GpSimd) rearranges columns into the packed
     band-major row order; bands grouped by equal width give affine 3D
     APs, one copy per (plane, width-group). Bias slots get sqrt(ssum).
  4. PE-transpose each 128-column block -> packed row chunks (XtB, f32r).
  5. Per band: 1-2 float32r matmuls (K=4fn+1, M=128 tokens, N=384).
  6. Evict PSUM->SBUF scaled by 1/sqrt(ssum) per token; DMA out.
"""

import math
import numpy as np

# ---------------- problem geometry (hardcoded, matches reference) ----------
SR, N_FFT, D = 44100, 2048, 384
RANGES = [(1000, 2), (2000, 4), (4000, 12), (8000, 24), (16000, 48)]


def _compute_bands(sr=SR, n_fft=N_FFT):
    hz_per_bin = sr / n_fft
    max_bin = n_fft // 2 + 1
    boundaries = [0]
    for hi_hz, bins in RANGES:
        hi_bin = math.floor(hi_hz / hz_per_bin)
        while boundaries[-1] + bins <= hi_bin and boundaries[-1] + bins <= max_bin:
            boundaries.append(boundaries[-1] + bins)
    if boundaries[-1] < max_bin:
        remaining = max_bin - boundaries[-1]
        step = math.ceil(remaining / 6)
        b = boundaries[-1]
        while b + step < max_bin:
            b += step
            boundaries.append(b)
        boundaries.append(max_bin)
    return [(boundaries[i], boundaries[i + 1]) for i in range(len(boundaries) - 1)]


BANDS = _compute_bands()
N_BANDS = len(BANDS)
assert N_BANDS == 62
FN = [r - l for l, r in BANDS]
MAXF = max(FN)
F_BINS = N_FFT // 2 + 1  # 1025
EPS = 1e-8
B, C, T = 8, 2, 512
TT = 128  # tokens per pass (matmul M)
NP = T // TT  # 4 passes
NPLANE = 4  # (c, ri) combinations, g = 2*c + ri

# ---------------- regular row layout by equal-width band groups ------------
# logical row j of band n: j=0 -> bias; j=1+g*fn+k -> plane g, freq l+k.
# Bands with equal fn are consecutive; within a group each band's rows
# start at G_base + i*pad, giving affine gather patterns.


def _pad_for(rows):
    for p in (32, 64, 128, 256):
        if rows <= p:
            return p
    raise AssertionError


def _plan():
    groups = []  # (n0, k, fn, l0, pad, gbase)
    rowbase = 0
    n = 0
    while n < N_BANDS:
        fn = FN[n]
        k = 1
        while n + k < N_BANDS and FN[n + k] == fn:
            k += 1
        rows = 1 + 4 * fn
        pad = _pad_for(rows)
        gbase = rowbase
        rowbase += ((k * pad + 127) // 128) * 128
        groups.append((n, k, fn, BANDS[n][0], pad, gbase))
        n += k
    nrows = rowbase  # multiple of 128
    nchunk = nrows // 128

    band_base = {}
    for (n0, k, fn, l0, pad, gbase) in groups:
        for i in range(k):
            band_base[n0 + i] = gbase + i * pad

    # matmul segments per band: (chunk, row0, klen) covering 1+4fn rows
    segs0 = []
    for n in range(N_BANDS):
        rows = 1 + 4 * FN[n]
        bb = band_base[n]
        out = []
        while rows > 0:
            ch, r0 = bb // 128, bb % 128
            kl = min(rows, 128 - r0)
            out.append((ch, r0, kl))
            bb += kl
            rows -= kl
        segs0.append(out)

    # W2 column blocks: greedy interval packing of (row0, row0+klen),
    # largest-first so full-height segments claim blocks before slivers.
    allsegs = []
    for n in range(N_BANDS):
        for si, (ch, r0, kl) in enumerate(segs0[n]):
            allsegs.append((kl, n, si, ch, r0))
    allsegs.sort(key=lambda x: -x[0])
    colblocks = []
    cb_of = {}
    for (kl, n, si, ch, r0) in allsegs:
        for cbi in range(len(colblocks) + 1):
            if cbi == len(colblocks):
                colblocks.append([])
            ivs = colblocks[cbi]
            if all(e <= r0 or s >= r0 + kl for (s, e) in ivs):
                ivs.append((r0, r0 + kl))
                cb_of[(n, si)] = cbi
                break
    ncolb = len(colblocks)
    segs = []
    for n in range(N_BANDS):
        segs.append(
            [
                (ch, r0, kl, cb_of[(n, si)])
                for si, (ch, r0, kl) in enumerate(segs0[n])
            ]
        )

    # host W2 packing map: logical row j of band n -> (w2row, colblock)
    w2map = []
    for n in range(N_BANDS):
        rows = 1 + 4 * FN[n]
        m = []
        j = 0
        for (ch, r0, kl, cb) in segs[n]:
            for q in range(kl):
                m.append((r0 + q, cb))
            j += kl
        assert j == rows
        w2map.append(m)
    return groups, nchunk, ncolb, segs, band_base, w2map


GROUPS, NCHUNK, NCOLB, SEGS, BAND_BASE, W2MAP = _plan()
NROWS = NCHUNK * 128

# ---------------- host-side constant prep ---------------------------------


def _prep_consts(norm_w, W, b):
    w2sb = np.zeros((128, NCOLB * D), np.float32)
    for n in range(N_BANDS):
        fn = FN[n]
        s = math.sqrt(4.0 * fn)
        row, cb = W2MAP[n][0]
        w2sb[row, cb * D:(cb + 1) * D] = b[n]
        w2rows = (s * norm_w[n][:, None] * W[n]).astype(np.float32)  # (216, 384)
        for g in range(NPLANE):
            for k in range(fn):
                row, cb = W2MAP[n][1 + g * fn + k]
                w2sb[row, cb * D:(cb + 1) * D] = w2rows[g * MAXF + k]
    idt = np.eye(128, dtype=np.float32)
    epsc = np.broadcast_to(
        (4.0 * np.asarray(FN, np.float64) * EPS).astype(np.float32)[None, :],
        (128, N_BANDS),
    ).copy()
    return w2sb, idt, epsc


# ---------------- bass kernel builder -------------------------------------

_BUILT = {}


def _build():
    if "nc" in _BUILT:
        return _BUILT["nc"]
    from contextlib import ExitStack
    import concourse.bacc as bacc
    import concourse.mybir as mybir
    from concourse import tile

    f32 = mybir.dt.float32
    f32r = mybir.dt.float32r

    nc = bacc.Bacc(None, target_bir_lowering=False)
    x_re = nc.declare_dram_parameter("X_real", [C, T, F_BINS], f32, isOutput=False)
    x_im = nc.declare_dram_parameter("X_imag", [C, T, F_BINS], f32, isOutput=False)
    w2_e = nc.declare_dram_parameter("W2", [128, NCOLB * D], f32, isOutput=False)
    id_e = nc.declare_dram_parameter("IDT", [128, 128], f32, isOutput=False)
    eps_e = nc.declare_dram_parameter("EPSC", [128, N_BANDS], f32, isOutput=False)
    out_e = nc.declare_dram_parameter("out", [T, N_BANDS, D], f32, isOutput=True)

    GSZ = 4   # bands per output staging group
    WLD = 2   # W2 colblocks per staged load chunk

    with tile.TileContext(nc) as tc, ExitStack() as ctx:
        const = ctx.enter_context(tc.tile_pool(name="const", bufs=1))
        xtbp = ctx.enter_context(tc.tile_pool(name="xtbp", bufs=1))
        x4p = ctx.enter_context(tc.tile_pool(name="x4p", bufs=2))
        wrp = ctx.enter_context(tc.tile_pool(name="wrp", bufs=2))
        msp = ctx.enter_context(tc.tile_pool(name="msv", bufs=2))
        scr = ctx.enter_context(tc.tile_pool(name="scr", bufs=2))
        spool = ctx.enter_context(tc.tile_pool(name="stagep", bufs=3))
        trps = ctx.enter_context(tc.tile_pool(name="trp", bufs=4, space="PSUM"))
        mmps = ctx.enter_context(tc.tile_pool(name="mmp", bufs=4, space="PSUM"))

        idsb = const.tile([128, 128], f32)
        nc.sync.dma_start(out=idsb[:], in_=id_e[:])
        epsc = const.tile([128, N_BANDS], f32)
        nc.sync.dma_start(out=epsc[:], in_=eps_e[:])
        w2sb = const.tile([128, NCOLB * D], f32r)
        for wb in range(0, NCOLB, WLD):
            wn = min(WLD, NCOLB - wb)
            wstage = wrp.tile([128, WLD * D], f32, tag="wstage")
            nc.sync.dma_start(
                out=wstage[:, 0:wn * D], in_=w2_e[:, wb * D:(wb + wn) * D]
            )
            nc.vector.tensor_copy(
                w2sb[:, wb * D:(wb + wn) * D], wstage[:, 0:wn * D]
            )

        xcat = const.tile([128, NROWS], f32, name="xcat0", tag="xcat0")
        nc.gpsimd.memset(xcat[:], 0.0)
        xtb = [
            xtbp.tile([128, TT], f32r, name=f"xtb{m}", tag=f"xtb{m}")
            for m in range(NCHUNK)
        ]

        for ps in range(NP):
            t0 = ps * TT
            xc = xcat
            x4 = x4p.tile([128, NPLANE, F_BINS], f32, tag="x4")
            for g in range(NPLANE):
                xsrc = x_re if g % 2 == 0 else x_im
                nc.sync.dma_start(
                    out=x4[:, g, :], in_=xsrc[g // 2, t0:t0 + TT, :]
                )
            # band sums: ssum[t, n] = sum over (plane, band freqs) x^2 + 4fn*eps
            ssum = msp.tile([128, N_BANDS], f32, tag="ssum")
            ssum_raw = msp.tile([128, N_BANDS], f32, tag="ssumr")
            sq_scr = scr.tile([128, NPLANE, MAXF], f32, tag="sqscr")
            for n in range(N_BANDS):
                l, r = BANDS[n]
                fn = FN[n]
                nc.vector.tensor_tensor(
                    out=sq_scr[:, :, 0:fn],
                    in0=x4[:, :, l:r],
                    in1=x4[:, :, l:r],
                    op=mybir.AluOpType.mult,
                )
                nc.vector.tensor_reduce(
                    out=ssum_raw[:, n:n + 1],
                    in_=sq_scr[:, :, 0:fn],
                    op=mybir.AluOpType.add,
                    axis=mybir.AxisListType.XY,
                )
            nc.vector.tensor_tensor(
                out=ssum[:], in0=ssum_raw[:], in1=epsc[:],
                op=mybir.AluOpType.add,
            )
            sqrt_n = msp.tile([128, N_BANDS], f32, tag="sqrtn")
            nc.scalar.activation(
                out=sqrt_n[:], in_=ssum[:], func=mybir.ActivationFunctionType.Sqrt
            )
            rs = msp.tile([128, N_BANDS], f32, tag="rs")
            nc.vector.reciprocal(rs[:], sqrt_n[:])

            # free-axis gather into packed row order (on GpSimd; plain f32)
            for (n0, k, fn, l0, pad, gbase) in GROUPS:
                for g in range(NPLANE):
                    src = x4[:, g, l0:l0 + k * fn].rearrange(
                        "p (k f) -> p k f", k=k
                    )
                    dst = xc[:, gbase:gbase + k * pad].rearrange(
                        "p (k q) -> p k q", k=k
                    )[:, :, 1 + g * fn:1 + (g + 1) * fn]
                    nc.vector.tensor_copy(dst, src)
                # bias slots: column 0 of each band <- sqrt(ssum)
                dstb = xc[:, gbase:gbase + k * pad].rearrange(
                    "p (k q) -> p k q", k=k
                )[:, :, 0:1]
                nc.vector.tensor_copy(
                    dstb, sqrt_n[:, n0:n0 + k].rearrange("p (k o) -> p k o", o=1)
                )

            # transpose all row chunks: (t, rows) -> (rows, t), cast to f32r
            for ch in range(NCHUNK):
                ptr = trps.tile([128, 128], f32, tag="trp")
                nc.tensor.transpose(
                    ptr[:], xc[:, ch * 128:(ch + 1) * 128], idsb[:]
                )
                nc.vector.tensor_copy(xtb[ch][:], ptr[:])

            # per-band matmuls + scaled eviction + output DMA
            for n0 in range(0, N_BANDS, GSZ):
                gn = min(GSZ, N_BANDS - n0)
                stage = spool.tile([128, GSZ * D], f32, tag="stage")
                for n in range(n0, n0 + gn):
                    pmm = mmps.tile([128, D], f32, tag="mmp")
                    nseg = len(SEGS[n])
                    for si, (ch, row0, klen, cb) in enumerate(SEGS[n]):
                        nc.tensor.matmul(
                            pmm[:],
                            lhsT=xtb[ch][row0:row0 + klen, :],
                            rhs=w2sb[row0:row0 + klen, cb * D:(cb + 1) * D],
                            start=(si == 0),
                            stop=(si == nseg - 1),
                            tile_position=(row0, 0),
                        )
                    slot = stage[:, (n - n0) * D:(n - n0 + 1) * D]
                    if n % 3 == 0:
                        nc.scalar.mul(slot, pmm[:], rs[:, n:n + 1])
                    else:
                        nc.vector.tensor_scalar_mul(slot, pmm[:], rs[:, n:n + 1])
                nc.sync.dma_start(
                    out=out_e[t0:t0 + TT, n0:n0 + gn, :],
                    in_=stage[:, 0:gn * D].rearrange("p (n d) -> p n d", n=gn),
                )

    nc.finalize()
    _BUILT["nc"] = nc
    return nc


# ---------------- entry points --------------------------------------------


def _run(in_maps, trace=False):
    from concourse.bass_utils import run_bass_kernel_spmd

    nc = _build()
    return run_bass_kernel_spmd(nc, in_maps, core_ids=list(range(8)), trace=trace)


def _run_traced(in_maps, tmpdir=None):
    from concourse.bass_utils import run_bass_kernel_spmd

    nc = _build()
    return run_bass_kernel_spmd(
        nc, in_maps, core_ids=list(range(8)), trace=True, tmpdir=tmpdir
    )


def _make_in_maps(X_real, X_imag, norm_w, W, b):
    X_real = np.ascontiguousarray(np.asarray(X_real, np.float32))
    X_imag = np.ascontiguousarray(np.asarray(X_imag, np.float32))
    w2sb, idt, epsc = _prep_consts(
        np.asarray(norm_w, np.float32), np.asarray(W, np.float32),
        np.asarray(b, np.float32),
    )
    return [
        {
            "X_real": X_real[i],
            "X_imag": X_imag[i],
            "W2": w2sb,
            "IDT": idt,
            "EPSC": epsc,
        }
        for i in range(B)
    ]


def kernel(X_real, X_imag, norm_w, W, b):
    res = _run(_make_in_maps(X_real, X_imag, norm_w, W, b), trace=False)
    return np.stack([res.results[i]["out"] for i in range(B)]).astype(np.float32)


def kernel_profiled(X_real, X_imag, norm_w, W, b):
    res = _run(_make_in_maps(X_real, X_imag, norm_w, W, b), trace=True)
    out = np.stack([res.results[i]["out"] for i in range(B)]).astype(np.float32)
    return out, res


if __name__ == "__main__":
    print(f"NCHUNK={NCHUNK} NCOLB={NCOLB} NROWS={NROWS}")
    print(f"groups: {[(g[0], g[1], g[2], g[4]) for g in GROUPS]}")
    print(f"matmul segs per pass: {sum(len(s) for s in SEGS)}")
    per_part = (2 * NROWS * 4 + NCHUNK * TT * 4 + NCOLB * D * 4
                + 2 * NPLANE * F_BINS * 4 + 2 * 4 * D * 4 + 3 * 4 * D * 4) / 1024
    print(f"approx SBUF per partition: {per_part:.0f} KB")


# revision 20
# speedup vs baseline: 17.9288x; 1.1555x over previous
"""BandSplit Trainium2 kernel: 8-core data-parallel over batch.

out[b,t,n,d] = rsqrt(ms + eps) * (x_band @ (norm_w * W)) + bias
with ms = sum(x_band^2)/(4*fn),  x_band = contiguous freq slices of X.

Math restructure (exact):
  rsqrt(ms + eps) = sqrt(4fn) / sqrt(ssum),  ssum = sum(x^2) + 4*fn*eps
  out = (1/sqrt(ssum)) * (x @ W2 + sqrt(ssum) * b)     [bias-row trick]
  where W2[n,p,:] = sqrt(4fn) * norm_w[n,p] * W[n,p,:]

Per core (one batch element, T=512 tokens, 4 passes of 128):
  1. DMA X planes into natural layout (t part, f free).
  2. ssum per band via one fused multiply-reduce per band (eps as initial).
  3. Free-axis gather (on GpSimd) rearranges columns into the packed
     band-major row order; bands grouped by equal width give affine 3D
     APs, one copy per (plane, width-group). Bias slots get sqrt(ssum).
  4. PE-transpose each 128-column block -> packed row chunks (XtB, f32r).
  5. Per band: 1-2 float32r matmuls (K=4fn+1, M=128 tokens, N=384).
  6. Evict PSUM->SBUF scaled by 1/sqrt(ssum) per token; DMA out.
"""

import math
import numpy as np

# ---------------- problem geometry (hardcoded, matches reference) ----------
SR, N_FFT, D = 44100, 2048, 384
RANGES = [(1000, 2), (2000, 4), (4000, 12), (8000, 24), (16000, 48)]


def _compute_bands(sr=SR, n_fft=N_FFT):
    hz_per_bin = sr / n_fft
    max_bin = n_fft // 2 + 1
    boundaries = [0]
    for hi_hz, bins in RANGES:
        hi_bin = math.floor(hi_hz / hz_per_bin)
        while boundaries[-1] + bins <= hi_bin and boundaries[-1] + bins <= max_bin:
            boundaries.append(boundaries[-1] + bins)
    if boundaries[-1] < max_bin:
        remaining = max_bin - boundaries[-1]
        step = math.ceil(remaining / 6)
        b = boundaries[-1]
        while b + step < max_bin:
            b += step
            boundaries.append(b)
        boundaries.append(max_bin)
    return [(boundaries[i], boundaries[i + 1]) for i in range(len(boundaries) - 1)]


BANDS = _compute_bands()
N_BANDS = len(BANDS)
assert N_BANDS == 62
FN = [r - l for l, r in BANDS]
MAXF = max(FN)
F_BINS = N_FFT // 2 + 1  # 1025
EPS = 1e-8
B, C, T = 8, 2, 512
TT = 128  # tokens per pass (matmul M)
NP = T // TT  # 4 passes
NPLANE = 4  # (c, ri) combinations, g = 2*c + ri

# ---------------- regular row layout by equal-width band groups ------------
# logical row j of band n: j=0 -> bias; j=1+g*fn+k -> plane g, freq l+k.
# Bands with equal fn are consecutive; within a group each band's rows
# start at G_base + i*pad, giving affine gather patterns.


def _pad_for(rows):
    for p in (32, 64, 128, 256):
        if rows <= p:
            return p
    raise AssertionError


def _plan():
    groups = []  # (n0, k, fn, l0, pad, gbase)
    rowbase = 0
    n = 0
    while n < N_BANDS:
        fn = FN[n]
        k = 1
        while n + k < N_BANDS and FN[n + k] == fn:
            k += 1
        rows = 1 + 4 * fn
        pad = _pad_for(rows)
        gbase = rowbase
        rowbase += ((k * pad + 127) // 128) * 128
        groups.append((n, k, fn, BANDS[n][0], pad, gbase))
        n += k
    nrows = rowbase  # multiple of 128
    nchunk = nrows // 128

    band_base = {}
    for (n0, k, fn, l0, pad, gbase) in groups:
        for i in range(k):
            band_base[n0 + i] = gbase + i * pad

    # matmul segments per band: (chunk, row0, klen) covering 1+4fn rows
    segs0 = []
    for n in range(N_BANDS):
        rows = 1 + 4 * FN[n]
        bb = band_base[n]
        out = []
        while rows > 0:
            ch, r0 = bb // 128, bb % 128
            kl = min(rows, 128 - r0)
            out.append((ch, r0, kl))
            bb += kl
            rows -= kl
        segs0.append(out)

    # W2 column blocks: greedy interval packing of (row0, row0+klen),
    # largest-first so full-height segments claim blocks before slivers.
    allsegs = []
    for n in range(N_BANDS):
        for si, (ch, r0, kl) in enumerate(segs0[n]):
            allsegs.append((kl, n, si, ch, r0))
    allsegs.sort(key=lambda x: -x[0])
    colblocks = []
    cb_of = {}
    for (kl, n, si, ch, r0) in allsegs:
        for cbi in range(len(colblocks) + 1):
            if cbi == len(colblocks):
                colblocks.append([])
            ivs = colblocks[cbi]
            if all(e <= r0 or s >= r0 + kl for (s, e) in ivs):
                ivs.append((r0, r0 + kl))
                cb_of[(n, si)] = cbi
                break
    ncolb = len(colblocks)
    segs = []
    for n in range(N_BANDS):
        segs.append(
            [
                (ch, r0, kl, cb_of[(n, si)])
                for si, (ch, r0, kl) in enumerate(segs0[n])
            ]
        )

    # host W2 packing map: logical row j of band n -> (w2row, colblock)
    w2map = []
    for n in range(N_BANDS):
        rows = 1 + 4 * FN[n]
        m = []
        j = 0
        for (ch, r0, kl, cb) in segs[n]:
            for q in range(kl):
                m.append((r0 + q, cb))
            j += kl
        assert j == rows
        w2map.append(m)
    return groups, nchunk, ncolb, segs, band_base, w2map


GROUPS, NCHUNK, NCOLB, SEGS, BAND_BASE, W2MAP = _plan()
NROWS = NCHUNK * 128

# ---------------- host-side constant prep ---------------------------------


def _prep_consts(norm_w, W, b):
    w2sb = np.zeros((128, NCOLB * D), np.float32)
    for n in range(N_BANDS):
        fn = FN[n]
        s = math.sqrt(4.0 * fn)
        row, cb = W2MAP[n][0]
        w2sb[row, cb * D:(cb + 1) * D] = b[n]
        w2rows = (s * norm_w[n][:, None] * W[n]).astype(np.float32)  # (216, 384)
        for g in range(NPLANE):
            for k in range(fn):
                row, cb = W2MAP[n][1 + g * fn + k]
                w2sb[row, cb * D:(cb + 1) * D] = w2rows[g * MAXF + k]
    idt = np.eye(128, dtype=np.float32)
    epsc = np.broadcast_to(
        (4.0 * np.asarray(FN, np.float64) * EPS).astype(np.float32)[None, :],
        (128, N_BANDS),
    ).copy()
    return w2sb, idt, epsc


# ---------------- bass kernel builder -------------------------------------

_BUILT = {}


def _build():
    if "nc" in _BUILT:
        return _BUILT["nc"]
    from contextlib import ExitStack
    import concourse.bacc as bacc
    import concourse.mybir as mybir
    from concourse import tile

    f32 = mybir.dt.float32
    f32r = mybir.dt.float32r

    nc = bacc.Bacc(None, target_bir_lowering=False)
    x_re = nc.declare_dram_parameter("X_real", [C, T, F_BINS], f32, isOutput=False)
    x_im = nc.declare_dram_parameter("X_imag", [C, T, F_BINS], f32, isOutput=False)
    w2_e = nc.declare_dram_parameter("W2", [128, NCOLB * D], f32, isOutput=False)
    id_e = nc.declare_dram_parameter("IDT", [128, 128], f32, isOutput=False)
    eps_e = nc.declare_dram_parameter("EPSC", [128, N_BANDS], f32, isOutput=False)
    out_e = nc.declare_dram_parameter("out", [T, N_BANDS, D], f32, isOutput=True)

    GSZ = 4   # bands per output staging group
    WLD = 2   # W2 colblocks per staged load chunk

    with tile.TileContext(nc) as tc, ExitStack() as ctx:
        const = ctx.enter_context(tc.tile_pool(name="const", bufs=1))
        xtbp = ctx.enter_context(tc.tile_pool(name="xtbp", bufs=1))
        x4p = ctx.enter_context(tc.tile_pool(name="x4p", bufs=2))
        wrp = ctx.enter_context(tc.tile_pool(name="wrp", bufs=2))
        msp = ctx.enter_context(tc.tile_pool(name="msv", bufs=2))
        scr = ctx.enter_context(tc.tile_pool(name="scr", bufs=2))
        spool = ctx.enter_context(tc.tile_pool(name="stagep", bufs=3))
        trps = ctx.enter_context(tc.tile_pool(name="trp", bufs=4, space="PSUM"))
        mmps = ctx.enter_context(tc.tile_pool(name="mmp", bufs=4, space="PSUM"))

        idsb = const.tile([128, 128], f32)
        nc.sync.dma_start(out=idsb[:], in_=id_e[:])
        epsc = const.tile([128, N_BANDS], f32)
        nc.sync.dma_start(out=epsc[:], in_=eps_e[:])
        w2sb = const.tile([128, NCOLB * D], f32r)
        for wb in range(0, NCOLB, WLD):
            wn = min(WLD, NCOLB - wb)
            wstage = wrp.tile([128, WLD * D], f32, tag="wstage")
            nc.sync.dma_start(
                out=wstage[:, 0:wn * D], in_=w2_e[:, wb * D:(wb + wn) * D]
            )
            nc.vector.tensor_copy(
                w2sb[:, wb * D:(wb + wn) * D], wstage[:, 0:wn * D]
            )

        xcat = const.tile([128, NROWS], f32, name="xcat0", tag="xcat0")
        nc.gpsimd.memset(xcat[:], 0.0)
        xtb = [
            xtbp.tile([128, TT], f32r, name=f"xtb{m}", tag=f"xtb{m}")
            for m in range(NCHUNK)
        ]

        for ps in range(NP):
            t0 = ps * TT
            xc = xcat
            x4 = x4p.tile([128, NPLANE, F_BINS], f32, tag="x4")
            for g in range(NPLANE):
                xsrc = x_re if g % 2 == 0 else x_im
                nc.sync.dma_start(
                    out=x4[:, g, :], in_=xsrc[g // 2, t0:t0 + TT, :]
                )
            # band sums: ssum[t, n] = sum over (plane, band freqs) x^2 + 4fn*eps
            ssum = msp.tile([128, N_BANDS], f32, tag="ssum")
            ssum_raw = msp.tile([128, N_BANDS], f32, tag="ssumr")
            qsum = scr.tile([128, F_BINS], f32, tag="qsum")
            sq_b = scr.tile([128, F_BINS], f32, tag="sqb")
            nc.vector.tensor_tensor(
                out=qsum[:], in0=x4[:, 0, :], in1=x4[:, 0, :],
                op=mybir.AluOpType.mult,
            )
            for g in range(1, NPLANE):
                nc.vector.tensor_tensor(
                    out=sq_b[:], in0=x4[:, g, :], in1=x4[:, g, :],
                    op=mybir.AluOpType.mult,
                )
                nc.vector.tensor_tensor(
                    out=qsum[:], in0=qsum[:], in1=sq_b[:],
                    op=mybir.AluOpType.add,
                )
            for n in range(N_BANDS):
                l, r = BANDS[n]
                nc.vector.tensor_reduce(
                    out=ssum_raw[:, n:n + 1],
                    in_=qsum[:, l:r],
                    op=mybir.AluOpType.add,
                    axis=mybir.AxisListType.X,
                )
            nc.vector.tensor_tensor(
                out=ssum[:], in0=ssum_raw[:], in1=epsc[:],
                op=mybir.AluOpType.add,
            )
            sqrt_n = msp.tile([128, N_BANDS], f32, tag="sqrtn")
            nc.scalar.activation(
                out=sqrt_n[:], in_=ssum[:], func=mybir.ActivationFunctionType.Sqrt
            )
            rs = msp.tile([128, N_BANDS], f32, tag="rs")
            nc.vector.reciprocal(rs[:], sqrt_n[:])

            # free-axis gather into packed row order (on GpSimd; plain f32)
            for (n0, k, fn, l0, pad, gbase) in GROUPS:
                for g in range(NPLANE):
                    src = x4[:, g, l0:l0 + k * fn].rearrange(
                        "p (k f) -> p k f", k=k
                    )
                    dst = xc[:, gbase:gbase + k * pad].rearrange(
                        "p (k q) -> p k q", k=k
                    )[:, :, 1 + g * fn:1 + (g + 1) * fn]
                    nc.gpsimd.tensor_copy(dst, src)
                # bias slots: column 0 of each band <- sqrt(ssum)
                dstb = xc[:, gbase:gbase + k * pad].rearrange(
                    "p (k q) -> p k q", k=k
                )[:, :, 0:1]
                nc.gpsimd.tensor_copy(
                    dstb, sqrt_n[:, n0:n0 + k].rearrange("p (k o) -> p k o", o=1)
                )

            # transpose all row chunks: (t, rows) -> (rows, t), cast to f32r
            for ch in range(NCHUNK):
                ptr = trps.tile([128, 128], f32, tag="trp")
                nc.tensor.transpose(
                    ptr[:], xc[:, ch * 128:(ch + 1) * 128], idsb[:]
                )
                if ch % 2 == 0:
                    nc.vector.tensor_copy(xtb[ch][:], ptr[:])
                else:
                    nc.scalar.copy(xtb[ch][:], ptr[:])

            # per-band matmuls + scaled eviction + output DMA
            for n0 in range(0, N_BANDS, GSZ):
                gn = min(GSZ, N_BANDS - n0)
                stage = spool.tile([128, GSZ * D], f32, tag="stage")
                for n in range(n0, n0 + gn):
                    pmm = mmps.tile([128, D], f32, tag="mmp")
                    nseg = len(SEGS[n])
                    for si, (ch, row0, klen, cb) in enumerate(SEGS[n]):
                        nc.tensor.matmul(
                            pmm[:],
                            lhsT=xtb[ch][row0:row0 + klen, :],
                            rhs=w2sb[row0:row0 + klen, cb * D:(cb + 1) * D],
                            start=(si == 0),
                            stop=(si == nseg - 1),
                            tile_position=(row0, 0),
                        )
                    slot = stage[:, (n - n0) * D:(n - n0 + 1) * D]
                    if n % 2 == 0:
                        nc.scalar.mul(slot, pmm[:], rs[:, n:n + 1])
                    else:
                        nc.vector.tensor_scalar_mul(slot, pmm[:], rs[:, n:n + 1])
                nc.sync.dma_start(
                    out=out_e[t0:t0 + TT, n0:n0 + gn, :],
                    in_=stage[:, 0:gn * D].rearrange("p (n d) -> p n d", n=gn),
                )

    nc.finalize()
    _BUILT["nc"] = nc
    return nc


# ---------------- entry points --------------------------------------------


def _run(in_maps, trace=False):
    from concourse.bass_utils import run_bass_kernel_spmd

    nc = _build()
    return run_bass_kernel_spmd(nc, in_maps, core_ids=list(range(8)), trace=trace)


def _run_traced(in_maps, tmpdir=None):
    from concourse.bass_utils import run_bass_kernel_spmd

    nc = _build()
    return run_bass_kernel_spmd(
        nc, in_maps, core_ids=list(range(8)), trace=True, tmpdir=tmpdir
    )


def _make_in_maps(X_real, X_imag, norm_w, W, b):
    X_real = np.ascontiguousarray(np.asarray(X_real, np.float32))
    X_imag = np.ascontiguousarray(np.asarray(X_imag, np.float32))
    w2sb, idt, epsc = _prep_consts(
        np.asarray(norm_w, np.float32), np.asarray(W, np.float32),
        np.asarray(b, np.float32),
    )
    return [
        {
            "X_real": X_real[i],
            "X_imag": X_imag[i],
            "W2": w2sb,
            "IDT": idt,
            "EPSC": epsc,
        }
        for i in range(B)
    ]


def kernel(X_real, X_imag, norm_w, W, b):
    res = _run(_make_in_maps(X_real, X_imag, norm_w, W, b), trace=False)
    return np.stack([res.results[i]["out"] for i in range(B)]).astype(np.float32)


def kernel_profiled(X_real, X_imag, norm_w, W, b):
    res = _run(_make_in_maps(X_real, X_imag, norm_w, W, b), trace=True)
    out = np.stack([res.results[i]["out"] for i in range(B)]).astype(np.float32)
    return out, res


if __name__ == "__main__":
    print(f"NCHUNK={NCHUNK} NCOLB={NCOLB} NROWS={NROWS}")
    print(f"groups: {[(g[0], g[1], g[2], g[4]) for g in GROUPS]}")
    print(f"matmul segs per pass: {sum(len(s) for s in SEGS)}")
    per_part = (2 * NROWS * 4 + NCHUNK * TT * 4 + NCOLB * D * 4
                + 2 * NPLANE * F_BINS * 4 + 2 * 4 * D * 4 + 3 * 4 * D * 4) / 1024
    print(f"approx SBUF per partition: {per_part:.0f} KB")


# revision 21
# speedup vs baseline: 18.8516x; 1.0515x over previous
"""BandSplit Trainium2 kernel: 8-core data-parallel over batch.

out[b,t,n,d] = rsqrt(ms + eps) * (x_band @ (norm_w * W)) + bias
with ms = sum(x_band^2)/(4*fn),  x_band = contiguous freq slices of X.

Math restructure (exact):
  rsqrt(ms + eps) = sqrt(4fn) / sqrt(ssum),  ssum = sum(x^2) + 4*fn*eps
  out = (1/sqrt(ssum)) * (x @ W2 + sqrt(ssum) * b)     [bias-row trick]
  where W2[n,p,:] = sqrt(4fn) * norm_w[n,p] * W[n,p,:]

Per core (one batch element, T=512 tokens, 4 passes of 128):
  1. DMA X planes into natural layout (t part, f free).
  2. ssum per band via one fused multiply-reduce per band (eps as initial).
  3. Free-axis gather (on GpSimd) rearranges columns into the packed
     band-major row order; bands grouped by equal width give affine 3D
     APs, one copy per (plane, width-group). Bias slots get sqrt(ssum).
  4. PE-transpose each 128-column block -> packed row chunks (XtB, f32r).
  5. Per band: 1-2 float32r matmuls (K=4fn+1, M=128 tokens, N=384).
  6. Evict PSUM->SBUF scaled by 1/sqrt(ssum) per token; DMA out.
"""

import math
import numpy as np

# ---------------- problem geometry (hardcoded, matches reference) ----------
SR, N_FFT, D = 44100, 2048, 384
RANGES = [(1000, 2), (2000, 4), (4000, 12), (8000, 24), (16000, 48)]


def _compute_bands(sr=SR, n_fft=N_FFT):
    hz_per_bin = sr / n_fft
    max_bin = n_fft // 2 + 1
    boundaries = [0]
    for hi_hz, bins in RANGES:
        hi_bin = math.floor(hi_hz / hz_per_bin)
        while boundaries[-1] + bins <= hi_bin and boundaries[-1] + bins <= max_bin:
            boundaries.append(boundaries[-1] + bins)
    if boundaries[-1] < max_bin:
        remaining = max_bin - boundaries[-1]
        step = math.ceil(remaining / 6)
        b = boundaries[-1]
        while b + step < max_bin:
            b += step
            boundaries.append(b)
        boundaries.append(max_bin)
    return [(boundaries[i], boundaries[i + 1]) for i in range(len(boundaries) - 1)]


BANDS = _compute_bands()
N_BANDS = len(BANDS)
assert N_BANDS == 62
FN = [r - l for l, r in BANDS]
MAXF = max(FN)
F_BINS = N_FFT // 2 + 1  # 1025
EPS = 1e-8
B, C, T = 8, 2, 512
TT = 128  # tokens per pass (matmul M)
NP = T // TT  # 4 passes
NPLANE = 4  # (c, ri) combinations, g = 2*c + ri

# ---------------- regular row layout by equal-width band groups ------------
# logical row j of band n: j=0 -> bias; j=1+g*fn+k -> plane g, freq l+k.
# Bands with equal fn are consecutive; within a group each band's rows
# start at G_base + i*pad, giving affine gather patterns.


def _pad_for(rows):
    for p in (32, 64, 128, 256):
        if rows <= p:
            return p
    raise AssertionError


def _plan():
    groups = []  # (n0, k, fn, l0, pad, gbase)
    rowbase = 0
    n = 0
    while n < N_BANDS:
        fn = FN[n]
        k = 1
        while n + k < N_BANDS and FN[n + k] == fn:
            k += 1
        rows = 1 + 4 * fn
        pad = _pad_for(rows)
        gbase = rowbase
        rowbase += ((k * pad + 127) // 128) * 128
        groups.append((n, k, fn, BANDS[n][0], pad, gbase))
        n += k
    nrows = rowbase  # multiple of 128
    nchunk = nrows // 128

    band_base = {}
    for (n0, k, fn, l0, pad, gbase) in groups:
        for i in range(k):
            band_base[n0 + i] = gbase + i * pad

    # matmul segments per band: (chunk, row0, klen) covering 1+4fn rows
    segs0 = []
    for n in range(N_BANDS):
        rows = 1 + 4 * FN[n]
        bb = band_base[n]
        out = []
        while rows > 0:
            ch, r0 = bb // 128, bb % 128
            kl = min(rows, 128 - r0)
            out.append((ch, r0, kl))
            bb += kl
            rows -= kl
        segs0.append(out)

    # W2 column blocks: greedy interval packing of (row0, row0+klen),
    # largest-first so full-height segments claim blocks before slivers.
    allsegs = []
    for n in range(N_BANDS):
        for si, (ch, r0, kl) in enumerate(segs0[n]):
            allsegs.append((kl, n, si, ch, r0))
    allsegs.sort(key=lambda x: -x[0])
    colblocks = []
    cb_of = {}
    for (kl, n, si, ch, r0) in allsegs:
        for cbi in range(len(colblocks) + 1):
            if cbi == len(colblocks):
                colblocks.append([])
            ivs = colblocks[cbi]
            if all(e <= r0 or s >= r0 + kl for (s, e) in ivs):
                ivs.append((r0, r0 + kl))
                cb_of[(n, si)] = cbi
                break
    ncolb = len(colblocks)
    segs = []
    for n in range(N_BANDS):
        segs.append(
            [
                (ch, r0, kl, cb_of[(n, si)])
                for si, (ch, r0, kl) in enumerate(segs0[n])
            ]
        )

    # host W2 packing map: logical row j of band n -> (w2row, colblock)
    w2map = []
    for n in range(N_BANDS):
        rows = 1 + 4 * FN[n]
        m = []
        j = 0
        for (ch, r0, kl, cb) in segs[n]:
            for q in range(kl):
                m.append((r0 + q, cb))
            j += kl
        assert j == rows
        w2map.append(m)
    return groups, nchunk, ncolb, segs, band_base, w2map


GROUPS, NCHUNK, NCOLB, SEGS, BAND_BASE, W2MAP = _plan()
NROWS = NCHUNK * 128
CHUNK_GROUP = []
for ch in range(NCHUNK):
    gi = max(i for i, g in enumerate(GROUPS) if g[5] <= ch * 128)
    CHUNK_GROUP.append(gi)

# ---------------- host-side constant prep ---------------------------------


def _prep_consts(norm_w, W, b):
    w2sb = np.zeros((128, NCOLB * D), np.float32)
    for n in range(N_BANDS):
        fn = FN[n]
        s = math.sqrt(4.0 * fn)
        row, cb = W2MAP[n][0]
        w2sb[row, cb * D:(cb + 1) * D] = b[n]
        w2rows = (s * norm_w[n][:, None] * W[n]).astype(np.float32)  # (216, 384)
        for g in range(NPLANE):
            for k in range(fn):
                row, cb = W2MAP[n][1 + g * fn + k]
                w2sb[row, cb * D:(cb + 1) * D] = w2rows[g * MAXF + k]
    idt = np.eye(128, dtype=np.float32)
    epsc = np.broadcast_to(
        (4.0 * np.asarray(FN, np.float64) * EPS).astype(np.float32)[None, :],
        (128, N_BANDS),
    ).copy()
    return w2sb, idt, epsc


# ---------------- bass kernel builder -------------------------------------

_BUILT = {}


def _build():
    if "nc" in _BUILT:
        return _BUILT["nc"]
    from contextlib import ExitStack
    import concourse.bacc as bacc
    import concourse.mybir as mybir
    from concourse import tile

    f32 = mybir.dt.float32
    f32r = mybir.dt.float32r

    nc = bacc.Bacc(None, target_bir_lowering=False)
    x_re = nc.declare_dram_parameter("X_real", [C, T, F_BINS], f32, isOutput=False)
    x_im = nc.declare_dram_parameter("X_imag", [C, T, F_BINS], f32, isOutput=False)
    w2_e = nc.declare_dram_parameter("W2", [128, NCOLB * D], f32, isOutput=False)
    id_e = nc.declare_dram_parameter("IDT", [128, 128], f32, isOutput=False)
    eps_e = nc.declare_dram_parameter("EPSC", [128, N_BANDS], f32, isOutput=False)
    out_e = nc.declare_dram_parameter("out", [T, N_BANDS, D], f32, isOutput=True)

    GSZ = 4   # bands per output staging group
    WLD = 2   # W2 colblocks per staged load chunk

    with tile.TileContext(nc) as tc, ExitStack() as ctx:
        const = ctx.enter_context(tc.tile_pool(name="const", bufs=1))
        xtbp = ctx.enter_context(tc.tile_pool(name="xtbp", bufs=1))
        x4p = ctx.enter_context(tc.tile_pool(name="x4p", bufs=2))
        wrp = ctx.enter_context(tc.tile_pool(name="wrp", bufs=2))
        msp = ctx.enter_context(tc.tile_pool(name="msv", bufs=2))
        scr = ctx.enter_context(tc.tile_pool(name="scr", bufs=2))
        spool = ctx.enter_context(tc.tile_pool(name="stagep", bufs=3))
        trps = ctx.enter_context(tc.tile_pool(name="trp", bufs=4, space="PSUM"))
        mmps = ctx.enter_context(tc.tile_pool(name="mmp", bufs=4, space="PSUM"))

        idsb = const.tile([128, 128], f32)
        nc.sync.dma_start(out=idsb[:], in_=id_e[:])
        epsc = const.tile([128, N_BANDS], f32)
        nc.sync.dma_start(out=epsc[:], in_=eps_e[:])
        w2sb = const.tile([128, NCOLB * D], f32r)
        for wb in range(0, NCOLB, WLD):
            wn = min(WLD, NCOLB - wb)
            wstage = wrp.tile([128, WLD * D], f32, tag="wstage")
            nc.sync.dma_start(
                out=wstage[:, 0:wn * D], in_=w2_e[:, wb * D:(wb + wn) * D]
            )
            nc.vector.tensor_copy(
                w2sb[:, wb * D:(wb + wn) * D], wstage[:, 0:wn * D]
            )

        xcat = []
        for gi, (n0, k, fn, l0, pad, gbase) in enumerate(GROUPS):
            gw = GROUPS[gi + 1][5] - gbase if gi + 1 < len(GROUPS) else NROWS - gbase
            xg = const.tile([128, gw], f32, name=f"xcat{gi}", tag=f"xcat{gi}")
            xcat.append(xg)
            nc.gpsimd.memset(xg[:], 0.0)
        xtb = [
            xtbp.tile([128, TT], f32r, name=f"xtb{m}", tag=f"xtb{m}")
            for m in range(NCHUNK)
        ]

        for ps in range(NP):
            t0 = ps * TT
            x4 = x4p.tile([128, NPLANE, F_BINS], f32, tag="x4")
            for g in range(NPLANE):
                xsrc = x_re if g % 2 == 0 else x_im
                nc.sync.dma_start(
                    out=x4[:, g, :], in_=xsrc[g // 2, t0:t0 + TT, :]
                )
            # band sums: ssum[t, n] = sum over (plane, band freqs) x^2 + 4fn*eps
            ssum = msp.tile([128, N_BANDS], f32, tag="ssum")
            ssum_raw = msp.tile([128, N_BANDS], f32, tag="ssumr")
            qsum = scr.tile([128, F_BINS], f32, tag="qsum")
            sq_b = scr.tile([128, F_BINS], f32, tag="sqb")
            nc.vector.tensor_tensor(
                out=qsum[:], in0=x4[:, 0, :], in1=x4[:, 0, :],
                op=mybir.AluOpType.mult,
            )
            for g in range(1, NPLANE):
                nc.vector.tensor_tensor(
                    out=sq_b[:], in0=x4[:, g, :], in1=x4[:, g, :],
                    op=mybir.AluOpType.mult,
                )
                nc.vector.tensor_tensor(
                    out=qsum[:], in0=qsum[:], in1=sq_b[:],
                    op=mybir.AluOpType.add,
                )
            for (n0, k, fn, l0, pad, gbase) in GROUPS:
                nc.vector.tensor_reduce(
                    out=ssum_raw[:, n0:n0 + k].rearrange("p (k o) -> p k o", o=1),
                    in_=qsum[:, l0:l0 + k * fn].rearrange("p (k f) -> p k f", k=k),
                    op=mybir.AluOpType.add,
                    axis=mybir.AxisListType.X,
                )
            nc.vector.tensor_tensor(
                out=ssum[:], in0=ssum_raw[:], in1=epsc[:],
                op=mybir.AluOpType.add,
            )
            sqrt_n = msp.tile([128, N_BANDS], f32, tag="sqrtn")
            nc.scalar.activation(
                out=sqrt_n[:], in_=ssum[:], func=mybir.ActivationFunctionType.Sqrt
            )
            rs = msp.tile([128, N_BANDS], f32, tag="rs")
            nc.vector.reciprocal(rs[:], sqrt_n[:])

            # free-axis gather into packed row order (DVE; plain f32)
            for gi, (n0, k, fn, l0, pad, gbase) in enumerate(GROUPS):
                xg = xcat[gi]
                for g in range(NPLANE):
                    src = x4[:, g, l0:l0 + k * fn].rearrange(
                        "p (k f) -> p k f", k=k
                    )
                    dst = xg[:, 0:k * pad].rearrange(
                        "p (k q) -> p k q", k=k
                    )[:, :, 1 + g * fn:1 + (g + 1) * fn]
                    nc.vector.tensor_copy(dst, src)
                # bias slots: column 0 of each band <- sqrt(ssum)
                dstb = xg[:, 0:k * pad].rearrange(
                    "p (k q) -> p k q", k=k
                )[:, :, 0:1]
                nc.vector.tensor_copy(
                    dstb, sqrt_n[:, n0:n0 + k].rearrange("p (k o) -> p k o", o=1)
                )

            # transpose all row chunks: (t, rows) -> (rows, t), cast to f32r
            for ch in range(NCHUNK):
                gi = CHUNK_GROUP[ch]
                off = ch * 128 - GROUPS[gi][5]
                ptr = trps.tile([128, 128], f32, tag="trp")
                nc.tensor.transpose(
                    ptr[:], xcat[gi][:, off:off + 128], idsb[:]
                )
                if ch % 2 == 0:
                    nc.vector.tensor_copy(xtb[ch][:], ptr[:])
                else:
                    nc.scalar.copy(xtb[ch][:], ptr[:])

            # per-band matmuls + scaled eviction + output DMA
            for n0 in range(0, N_BANDS, GSZ):
                gn = min(GSZ, N_BANDS - n0)
                stage = spool.tile([128, GSZ * D], f32, tag="stage")
                for n in range(n0, n0 + gn):
                    pmm = mmps.tile([128, D], f32, tag="mmp")
                    nseg = len(SEGS[n])
                    for si, (ch, row0, klen, cb) in enumerate(SEGS[n]):
                        nc.tensor.matmul(
                            pmm[:],
                            lhsT=xtb[ch][row0:row0 + klen, :],
                            rhs=w2sb[row0:row0 + klen, cb * D:(cb + 1) * D],
                            start=(si == 0),
                            stop=(si == nseg - 1),
                            tile_position=(row0, 0),
                        )
                    slot = stage[:, (n - n0) * D:(n - n0 + 1) * D]
                    if n % 2 == 0:
                        nc.scalar.mul(slot, pmm[:], rs[:, n:n + 1])
                    else:
                        nc.vector.tensor_scalar_mul(slot, pmm[:], rs[:, n:n + 1])
                nc.sync.dma_start(
                    out=out_e[t0:t0 + TT, n0:n0 + gn, :],
                    in_=stage[:, 0:gn * D].rearrange("p (n d) -> p n d", n=gn),
                )

    nc.finalize()
    _BUILT["nc"] = nc
    return nc


# ---------------- entry points --------------------------------------------


def _run(in_maps, trace=False):
    from concourse.bass_utils import run_bass_kernel_spmd

    nc = _build()
    return run_bass_kernel_spmd(nc, in_maps, core_ids=list(range(8)), trace=trace)


def _run_traced(in_maps, tmpdir=None):
    from concourse.bass_utils import run_bass_kernel_spmd

    nc = _build()
    return run_bass_kernel_spmd(
        nc, in_maps, core_ids=list(range(8)), trace=True, tmpdir=tmpdir
    )


def _make_in_maps(X_real, X_imag, norm_w, W, b):
    X_real = np.ascontiguousarray(np.asarray(X_real, np.float32))
    X_imag = np.ascontiguousarray(np.asarray(X_imag, np.float32))
    w2sb, idt, epsc = _prep_consts(
        np.asarray(norm_w, np.float32), np.asarray(W, np.float32),
        np.asarray(b, np.float32),
    )
    return [
        {
            "X_real": X_real[i],
            "X_imag": X_imag[i],
            "W2": w2sb,
            "IDT": idt,
            "EPSC": epsc,
        }
        for i in range(B)
    ]


def kernel(X_real, X_imag, norm_w, W, b):
    res = _run(_make_in_maps(X_real, X_imag, norm_w, W, b), trace=False)
    return np.stack([res.results[i]["out"] for i in range(B)]).astype(np.float32)


def kernel_profiled(X_real, X_imag, norm_w, W, b):
    res = _run(_make_in_maps(X_real, X_imag, norm_w, W, b), trace=True)
    out = np.stack([res.results[i]["out"] for i in range(B)]).astype(np.float32)
    return out, res


if __name__ == "__main__":
    print(f"NCHUNK={NCHUNK} NCOLB={NCOLB} NROWS={NROWS}")
    print(f"groups: {[(g[0], g[1], g[2], g[4]) for g in GROUPS]}")
    print(f"matmul segs per pass: {sum(len(s) for s in SEGS)}")
    per_part = (2 * NROWS * 4 + NCHUNK * TT * 4 + NCOLB * D * 4
                + 2 * NPLANE * F_BINS * 4 + 2 * 4 * D * 4 + 3 * 4 * D * 4) / 1024
    print(f"approx SBUF per partition: {per_part:.0f} KB")
